# revision 13
# baseline (speedup 1.0000x reference)
"""CrissCrossAttention3D Trainium2 kernel.

B=2, C=512, CQK=64, H=W=D=32, 8 NeuronCores.

Three SPMD launches (same program on all 8 cores, per-core data via in_maps),
host numpy resharding between launches:

  L1 (voxel-sharded, 8192 voxels/core):
      q,k = Wqk @ x   (psum [128, n]),  vT[n, c] = x_chunk.T-stationary @ WvT
  L2 (d-slab + h-slab roles per core): per-line energy matmuls
      E[q, l] = Q_line.T @ K_line  (fp32), exp on ACT -> bf16, per-line
      partial sums via DVE reduce.  No masking on device (host fixes diag).
  L3: aggregation  out[q, c] = sum_l a[l, q] * vT[l, c]  with 4-line
      diagonal 32x32 PE tiling, softmax normalization (and gamma) fused
      into the PSUM-evacuation scale.

Host: builds line-ordered views, computes r = gamma / sum(exp), zeroes the
masked diagonals, does the final scatter-add  y = x + oH + oW + oD.
"""

import numpy as np
import ml_dtypes

import concourse.bass as bass
from concourse import bacc
import concourse.tile as tile
from concourse import mybir
from concourse.bass_utils import run_bass_kernel_spmd

BF16 = ml_dtypes.bfloat16
B, C, H, W, D = 2, 512, 32, 32, 32
CQK = 64
NCORES = 8
G = 4          # d-groups (and h-groups for the D-axis role)
DS = D // G    # 8 slab thickness
NV = 8192      # voxels per core in every launch
LINES = 256    # lines per axis per core (H/W: 32*8, D: 8*32)
PACKS = 64     # 4-line packs per axis

f32 = mybir.dt.float32
f32r = mybir.dt.float32r
bf16 = mybir.dt.bfloat16

_cache = {}


# --------------------------------------------------------------------------
# L1: projections
# --------------------------------------------------------------------------
def build_l1():
    nc = bacc.Bacc()
    x_in = nc.declare_dram_parameter("x", [4, 128, NV], f32r, isOutput=False)
    wqk_in = nc.declare_dram_parameter("wqk", [4, 128, 128], f32r, isOutput=False)
    wv_in = nc.declare_dram_parameter("wv", [4, 128, 512], f32r, isOutput=False)
    qk_out = nc.declare_dram_parameter("qk", [128, NV], f32, isOutput=True)
    vt_out = nc.declare_dram_parameter("vt", [64, 128, 512], bf16, isOutput=True)

    with tile.TileContext(nc) as tc:
        with (
            tc.tile_pool(name="w", bufs=1) as wpool,
            tc.tile_pool(name="xb", bufs=2) as xpool,
            tc.tile_pool(name="ev", bufs=4) as evpool,
            tc.tile_pool(name="ps", bufs=4, space="PSUM") as pspool,
        ):
            wqk_sb = wpool.tile([128, 512], f32r, tag="wqk")
            wv_sb = wpool.tile([128, 2048], f32r, tag="wv")
            for ci in range(4):
                nc.gpsimd.dma_start(wqk_sb[:, ci * 128:(ci + 1) * 128], wqk_in[ci])
                nc.gpsimd.dma_start(wv_sb[:, ci * 512:(ci + 1) * 512], wv_in[ci])

            for nb in range(16):  # 512-voxel blocks
                xt = xpool.tile([128, 2048], f32r, tag="x")
                for ci in range(4):
                    nc.gpsimd.dma_start(xt[:, ci * 512:(ci + 1) * 512],
                                        x_in[ci, :, nb * 512:(nb + 1) * 512])

                ps_qk = pspool.tile([128, 512], f32, tag="ps")
                for ci in range(4):
                    nc.tensor.matmul(ps_qk[:],
                                     wqk_sb[:, ci * 128:(ci + 1) * 128],
                                     xt[:, ci * 512:(ci + 1) * 512],
                                     start=(ci == 0), stop=(ci == 3))
                qk_sb = evpool.tile([128, 512], f32, tag="qk")
                nc.scalar.activation(qk_sb[:], ps_qk[:],
                                     mybir.ActivationFunctionType.Copy)
                nc.gpsimd.dma_start(qk_out[:, nb * 512:(nb + 1) * 512], qk_sb[:])

                for sub in range(4):  # 128-voxel sub-blocks -> vT
                    ps_v = pspool.tile([128, 512], f32, tag="ps")
                    for ci in range(4):
                        nc.tensor.matmul(
                            ps_v[:],
                            xt[:, ci * 512 + sub * 128:ci * 512 + (sub + 1) * 128],
                            wv_sb[:, ci * 512:(ci + 1) * 512],
                            start=(ci == 0), stop=(ci == 3))
                    v_sb = evpool.tile([128, 512], bf16, tag="v")
                    if sub % 2 == 0:
                        nc.scalar.activation(v_sb[:], ps_v[:],
                                             mybir.ActivationFunctionType.Copy)
                    else:
                        nc.vector.tensor_copy(v_sb[:], ps_v[:])
                    nc.gpsimd.dma_start(vt_out[nb * 4 + sub], v_sb[:])
    return nc


# --------------------------------------------------------------------------
# L2: energies + exp + per-line sums
# --------------------------------------------------------------------------
def build_l2():
    nc = bacc.Bacc()
    qs, ks, es, ss = {}, {}, {}, {}
    for ax in "hwd":
        qs[ax] = nc.declare_dram_parameter(f"q{ax}", [64, NV], f32, isOutput=False)
        ks[ax] = nc.declare_dram_parameter(f"k{ax}", [64, NV], f32, isOutput=False)
        es[ax] = nc.declare_dram_parameter(f"e{ax}", [128, 2048], bf16, isOutput=True)
        ss[ax] = nc.declare_dram_parameter(f"s{ax}", [128, 64], f32, isOutput=True)

    with tile.TileContext(nc) as tc:
        with (
            tc.tile_pool(name="qk", bufs=1) as qkpool,
            tc.tile_pool(name="ev", bufs=4) as evpool,
            tc.tile_pool(name="sm", bufs=1) as smpool,
            tc.tile_pool(name="ps", bufs=4, space="PSUM") as pspool,
        ):
            for ax in "hwd":
                q_sb = qkpool.tile([64, NV], f32, tag=f"q{ax}")
                k_sb = qkpool.tile([64, NV], f32, tag=f"k{ax}")
                nc.gpsimd.dma_start(q_sb[:], qs[ax][:])
                nc.gpsimd.dma_start(k_sb[:], ks[ax][:])
                s_sb = smpool.tile([128, 64], f32, tag=f"s{ax}")
                for bank in range(4):
                    ps = pspool.tile([128, 512], f32, tag="ps")
                    for q16 in range(16):
                        p = bank * 16 + q16
                        for j in range(4):
                            ln = 4 * p + j
                            nc.tensor.matmul(
                                ps[32 * j:32 * j + 32, q16 * 32:q16 * 32 + 32],
                                q_sb[:, ln * 32:ln * 32 + 32],
                                k_sb[:, ln * 32:ln * 32 + 32],
                                start=True, stop=True,
                                tile_position=(0, 32 * j))
                    e_sb = evpool.tile([128, 512], bf16, tag="e")
                    nc.scalar.activation(e_sb[:], ps[:],
                                         mybir.ActivationFunctionType.Exp)
                    nc.vector.tensor_reduce(
                        s_sb[:, bank * 16:bank * 16 + 16],
                        e_sb[:].rearrange("p (g l) -> p g l", l=32),
                        axis=mybir.AxisListType.X, op=mybir.AluOpType.add)
                    nc.gpsimd.dma_start(es[ax][:, bank * 512:(bank + 1) * 512], e_sb[:])
                nc.gpsimd.dma_start(ss[ax][:], s_sb[:])
    return nc


# --------------------------------------------------------------------------
# L3: aggregation with fused normalization
# --------------------------------------------------------------------------
def build_l3():
    nc = bacc.Bacc()
    as_, vs_, rs_, os_ = {}, {}, {}, {}
    for ax in "hwd":
        as_[ax] = nc.declare_dram_parameter(f"a{ax}", [128, 2048], bf16, isOutput=False)
        vs_[ax] = nc.declare_dram_parameter(f"v{ax}", [64, 128, 512], bf16, isOutput=False)
        rs_[ax] = nc.declare_dram_parameter(f"r{ax}", [128, 64], f32, isOutput=False)
        os_[ax] = nc.declare_dram_parameter(f"o{ax}", [64, 128, 512], bf16, isOutput=True)

    with tile.TileContext(nc) as tc:
        with (
            tc.tile_pool(name="aw", bufs=1) as apool,
            tc.tile_pool(name="vt", bufs=4) as vpool,
            tc.tile_pool(name="ev", bufs=4) as evpool,
            tc.tile_pool(name="ps", bufs=4, space="PSUM") as pspool,
        ):
            for ax in "hwd":
                a_sb = apool.tile([128, 2048], bf16, tag=f"a{ax}")
                r_sb = apool.tile([128, 64], f32, tag=f"r{ax}")
                nc.gpsimd.dma_start(a_sb[:], as_[ax][:])
                nc.gpsimd.dma_start(r_sb[:], rs_[ax][:])
                for p in range(PACKS):
                    v_sb = vpool.tile([128, 512], bf16, tag="v")
                    nc.gpsimd.dma_start(v_sb[:], vs_[ax][p])
                    ps = pspool.tile([128, 512], f32, tag="ps")
                    for j in range(4):
                        nc.tensor.matmul(
                            ps[32 * j:32 * j + 32, :],
                            a_sb[32 * j:32 * j + 32, p * 32:p * 32 + 32],
                            v_sb[32 * j:32 * j + 32, :],
                            start=True, stop=True,
                            tile_position=(32 * j, 32 * j))
                    o_sb = evpool.tile([128, 512], bf16, tag="o")
                    if p % 2 == 0:
                        nc.scalar.activation(o_sb[:], ps[:],
                                             mybir.ActivationFunctionType.Copy,
                                             scale=r_sb[:, p:p + 1])
                    else:
                        nc.vector.tensor_scalar_mul(o_sb[:], ps[:], r_sb[:, p:p + 1])
                    nc.gpsimd.dma_start(os_[ax][p], o_sb[:])
    return nc


def _get(name, builder):
    if name not in _cache:
        nc = builder()
        nc.finalize()
        _cache[name] = nc
    return _cache[name]


class _Runner:
    """jit-once PJRT runner for a prebuilt Bass module across 8 cores."""

    def __init__(self, nc):
        import jax
        from jax.experimental.shard_map import shard_map
        from jax.sharding import Mesh, PartitionSpec
        from concourse import bass2jax, mybir as _mb
        bass2jax.install_neuronx_cc_hook()
        self.nc = nc
        pname = nc.partition_id_tensor.name if nc.partition_id_tensor else None
        in_names, out_names, out_avals = [], [], []
        for alloc in nc.m.functions[0].allocations:
            if not isinstance(alloc, _mb.MemoryLocationSet):
                continue
            name = alloc.memorylocations[0].name
            if alloc.kind == "ExternalInput":
                if name != pname:
                    in_names.append(name)
            elif alloc.kind == "ExternalOutput":
                shape = tuple(alloc.tensor_shape)
                dt_np = _mb.dt.np(alloc.dtype)
                out_names.append(name)
                out_avals.append(jax.core.ShapedArray(shape, dt_np))
        self.in_names, self.out_names, self.out_avals = in_names, out_names, out_avals
        n_params = len(in_names)
        all_in = list(in_names) + list(out_names) + ([pname] if pname else [])

        def _body(*args):
            ops = list(args)
            if pname is not None:
                ops.append(bass2jax.partition_id_tensor())
            outs = bass2jax._bass_exec_p.bind(
                *ops, out_avals=tuple(out_avals), in_names=tuple(all_in),
                out_names=tuple(out_names), lowering_input_output_aliases=(),
                sim_require_finite=True, sim_require_nnan=True, nc=nc)
            return tuple(outs)

        devices = jax.devices()[:NCORES]
        mesh = Mesh(np.array(devices), ("core",))
        self.mesh = mesh
        n_io = n_params + len(out_names)
        self.donate = tuple(range(n_params, n_io))
        self.sharded = jax.jit(
            shard_map(_body, mesh=mesh,
                      in_specs=(PartitionSpec("core"),) * n_io,
                      out_specs=(PartitionSpec("core"),) * len(out_names),
                      check_rep=False),
            donate_argnums=self.donate, keep_unused=True)

    def _zeros(self):
        return [np.zeros((NCORES * a.shape[0], *a.shape[1:]), a.dtype)
                for a in self.out_avals]

    def __call__(self, in_maps):
        concat = [np.concatenate([np.asarray(m[n]) for m in in_maps], axis=0)
                  for n in self.in_names]
        arrs = self.sharded(*concat, *self._zeros())
        out = [{n: np.asarray(arrs[i]).reshape(NCORES, *self.out_avals[i].shape)[c]
                for i, n in enumerate(self.out_names)} for c in range(NCORES)]
        return out, (concat,)

    def bench(self, concat, n=3):
        import time, jax
        from jax.sharding import NamedSharding, PartitionSpec
        sh = NamedSharding(self.mesh, PartitionSpec("core"))
        dev_in = [jax.device_put(c, sh) for c in concat]
        for a in dev_in:
            a.block_until_ready()
        ts = []
        for _ in range(n):
            zs = [jax.device_put(z, sh) for z in self._zeros()]
            for z in zs:
                z.block_until_ready()
            t0 = time.perf_counter()
            arrs = self.sharded(*dev_in, *zs)
            for a in arrs:
                a.block_until_ready()
            ts.append(time.perf_counter() - t0)
        return min(ts)


class _RunRes:
    def __init__(self, results, exec_time_ns):
        self.results = results
        self.exec_time_ns = exec_time_ns


def _run(nc, in_maps, trace=False):
    import os
    key = id(nc)
    if key not in _cache:
        _cache[key] = _Runner(nc)
    runner = _cache[key]
    results, (concat,) = runner(in_maps)
    t = None
    if os.environ.get("BENCH"):
        t = int(runner.bench(concat, int(os.environ["BENCH"])) * 1e9)
    return _RunRes(results, t)


# --------------------------------------------------------------------------
# host orchestration
# --------------------------------------------------------------------------
def kernel(x, Wq, bq, Wk, bk, Wv, bv, gamma, _trace=False, _times=None):
    x = np.asarray(x, np.float32)
    Wq = np.asarray(Wq, np.float32); bq = np.asarray(bq, np.float32)
    Wk = np.asarray(Wk, np.float32); bk = np.asarray(bk, np.float32)
    Wv = np.asarray(Wv, np.float32); bv = np.asarray(bv, np.float32)
    gam = float(np.asarray(gamma))

    # ---------------- L1 ----------------
    wqk = np.concatenate([Wq.T, Wk.T], axis=1).reshape(4, 128, 128)
    wv = np.ascontiguousarray(Wv.T).reshape(4, 128, 512)
    in1 = []
    for core in range(NCORES):
        b, j = divmod(core, G)
        xc = x[b].reshape(C, H * W * D)[:, j * NV:(j + 1) * NV]
        in1.append({"x": np.ascontiguousarray(xc).reshape(4, 128, NV),
                    "wqk": wqk, "wv": wv})
    r1 = _run(_get("l1", build_l1), in1, trace=_trace)
    if _times is not None:
        _times.append(r1.exec_time_ns)

    q = np.empty((B, CQK, H * W * D), np.float32)
    k = np.empty((B, CQK, H * W * D), np.float32)
    vt = np.empty((B, H * W * D, 512), BF16)
    for core in range(NCORES):
        b, j = divmod(core, G)
        qk_c = r1.results[core]["qk"]
        q[b, :, j * NV:(j + 1) * NV] = qk_c[:64]
        k[b, :, j * NV:(j + 1) * NV] = qk_c[64:]
        vt[b, j * NV:(j + 1) * NV] = r1.results[core]["vt"].reshape(NV, 512)
    if bq.any():
        q += bq[None, :, None]
    if bk.any():
        k += bk[None, :, None]
    if bv.any():
        vt = (vt.astype(np.float32) + bv[None, None, :]).astype(BF16)

    # ---------------- L2 ----------------
    q4 = q.reshape(B, CQK, H, W, D)
    k4 = k.reshape(B, CQK, H, W, D)
    in2 = []
    for core in range(NCORES):
        b, g = divmod(core, G)
        sl = slice(g * DS, (g + 1) * DS)
        m = {}
        for nm, a4 in (("q", q4), ("k", k4)):
            m[nm + "h"] = np.ascontiguousarray(
                a4[b][:, :, :, sl].transpose(0, 2, 3, 1)).reshape(64, NV)
            m[nm + "w"] = np.ascontiguousarray(
                a4[b][:, :, :, sl].transpose(0, 1, 3, 2)).reshape(64, NV)
            m[nm + "d"] = np.ascontiguousarray(a4[b][:, sl]).reshape(64, NV)
        in2.append(m)
    r2 = _run(_get("l2", build_l2), in2, trace=_trace)
    if _times is not None:
        _times.append(r2.exec_time_ns)

    def dec_e(e):   # [128,2048] -> [256 lines, 32 q, 32 l]
        return np.ascontiguousarray(
            e.reshape(4, 32, 64, 32).transpose(2, 0, 1, 3)).reshape(LINES, 32, 32)

    def dec_s(s):   # [128,64] -> [256 lines, 32 q]
        return np.ascontiguousarray(
            s.reshape(4, 32, 64).transpose(2, 0, 1)).reshape(LINES, 32)

    ar = np.arange(32)
    E = {}          # (core, ax) -> masked exp energies [lines, q, l] float32
    sig = np.empty((B, H, W, D), np.float32)
    sig[:] = 0.0
    for core in range(NCORES):
        b, g = divmod(core, G)
        sl = slice(g * DS, (g + 1) * DS)
        for ax in "hwd":
            e = dec_e(r2.results[core][f"e{ax}"]).astype(np.float32)
            s = dec_s(r2.results[core][f"s{ax}"])
            if ax != "w":   # mask self: subtract diag from sums, zero diag
                s = s - e[:, ar, ar]
                e[:, ar, ar] = 0.0
            E[(core, ax)] = e
            if ax == "h":   # lines (w,dh), q=h
                sig[b, :, :, sl] += s.reshape(W, DS, 32).transpose(2, 0, 1)
            elif ax == "w":  # lines (h,dh), q=w
                sig[b, :, :, sl] += s.reshape(H, DS, 32).transpose(0, 2, 1)
            else:           # lines (h in slab, w), q=d
                sig[b, sl] += s.reshape(DS, W, 32)
    r = gam / sig   # [B, H, W, D]

    def pack_a(e):  # [lines, q, l] -> lhsT layout [128, 2048] bf16
        return np.ascontiguousarray(
            e.transpose(0, 2, 1).reshape(PACKS, 4, 32, 32)
            .transpose(1, 2, 0, 3)).reshape(128, 2048).astype(BF16)

    def pack_r(rv):  # [lines, q] -> [128, 64] f32
        return np.ascontiguousarray(
            rv.reshape(PACKS, 4, 32).transpose(1, 2, 0)).reshape(128, 64)

    vt4 = vt.reshape(B, H, W, D, 512)
    in3 = []
    for core in range(NCORES):
        b, g = divmod(core, G)
        sl = slice(g * DS, (g + 1) * DS)
        m = {}
        m["ah"] = pack_a(E[(core, "h")])
        m["aw"] = pack_a(E[(core, "w")])
        m["ad"] = pack_a(E[(core, "d")])
        m["rh"] = pack_r(np.ascontiguousarray(
            r[b][:, :, sl].transpose(1, 2, 0)).reshape(LINES, 32))
        m["rw"] = pack_r(np.ascontiguousarray(
            r[b][:, :, sl].transpose(0, 2, 1)).reshape(LINES, 32))
        m["rd"] = pack_r(r[b][sl].reshape(LINES, 32))
        m["vh"] = np.ascontiguousarray(
            vt4[b][:, :, sl].transpose(1, 2, 0, 3)).reshape(64, 128, 512)
        m["vw"] = np.ascontiguousarray(
            vt4[b][:, :, sl].transpose(0, 2, 1, 3)).reshape(64, 128, 512)
        m["vd"] = np.ascontiguousarray(vt4[b][sl]).reshape(64, 128, 512)
        in3.append(m)
    r3 = _run(_get("l3", build_l3), in3, trace=_trace)
    if _times is not None:
        _times.append(r3.exec_time_ns)

    # ---------------- final scatter-add ----------------
    acc = np.zeros((B, H, W, D, C), np.float32)
    for core in range(NCORES):
        b, g = divmod(core, G)
        sl = slice(g * DS, (g + 1) * DS)
        oh = r3.results[core]["oh"].astype(np.float32).reshape(PACKS, 4, 32, 512)
        ow = r3.results[core]["ow"].astype(np.float32).reshape(PACKS, 4, 32, 512)
        od = r3.results[core]["od"].astype(np.float32).reshape(PACKS, 4, 32, 512)
        # [pack, jj, q, c] -> [line, q, c]
        oh = oh.transpose(0, 1, 2, 3).reshape(LINES, 32, 512)
        ow = ow.reshape(LINES, 32, 512)
        od = od.reshape(LINES, 32, 512)
        acc[b][:, :, sl] += oh.reshape(W, DS, 32, 512).transpose(2, 0, 1, 3)
        acc[b][:, :, sl] += ow.reshape(H, DS, 32, 512).transpose(0, 2, 1, 3)
        acc[b][sl] += od.reshape(DS, W, 32, 512)
    y = x + acc.transpose(0, 4, 1, 2, 3)
    return y


# revision 14
# speedup vs baseline: 1.2587x; 1.2587x over previous
"""CrissCrossAttention3D Trainium2 kernel.

B=2, C=512, CQK=64, H=W=D=32, 8 NeuronCores.

Three SPMD launches (same program on all 8 cores, per-core data via in_maps),
host numpy resharding between launches:

  L1 (voxel-sharded, 8192 voxels/core):
      q,k = Wqk @ x   (psum [128, n]),  vT[n, c] = x_chunk.T-stationary @ WvT
  L2 (d-slab + h-slab roles per core): per-line energy matmuls
      E[q, l] = Q_line.T @ K_line  (fp32), exp on ACT -> bf16, per-line
      partial sums via DVE reduce.  No masking on device (host fixes diag).
  L3: aggregation  out[q, c] = sum_l a[l, q] * vT[l, c]  with 4-line
      diagonal 32x32 PE tiling, softmax normalization (and gamma) fused
      into the PSUM-evacuation scale.

Host: builds line-ordered views, computes r = gamma / sum(exp), zeroes the
masked diagonals, does the final scatter-add  y = x + oH + oW + oD.
"""

import numpy as np
import ml_dtypes

import concourse.bass as bass
from concourse import bacc
import concourse.tile as tile
from concourse import mybir
from concourse.bass_utils import run_bass_kernel_spmd

BF16 = ml_dtypes.bfloat16
B, C, H, W, D = 2, 512, 32, 32, 32
CQK = 64
NCORES = 8
G = 4          # d-groups (and h-groups for the D-axis role)
DS = D // G    # 8 slab thickness
NV = 8192      # voxels per core in every launch
LINES = 256    # lines per axis per core (H/W: 32*8, D: 8*32)
PACKS = 64     # 4-line packs per axis

f32 = mybir.dt.float32
f32r = mybir.dt.float32r
bf16 = mybir.dt.bfloat16

_cache = {}


# --------------------------------------------------------------------------
# L1: projections
# --------------------------------------------------------------------------
def build_l1():
    nc = bacc.Bacc()
    x_in = nc.declare_dram_parameter("x", [4, 128, NV], f32r, isOutput=False)
    wqk_in = nc.declare_dram_parameter("wqk", [4, 128, 128], f32r, isOutput=False)
    wv_in = nc.declare_dram_parameter("wv", [4, 128, 512], f32r, isOutput=False)
    qk_out = nc.declare_dram_parameter("qk", [128, NV], f32, isOutput=True)
    vt_out = nc.declare_dram_parameter("vt", [64, 128, 512], bf16, isOutput=True)

    with tile.TileContext(nc) as tc:
        with (
            tc.tile_pool(name="w", bufs=1) as wpool,
            tc.tile_pool(name="xb", bufs=2) as xpool,
            tc.tile_pool(name="ev", bufs=4) as evpool,
            tc.tile_pool(name="ps", bufs=4, space="PSUM") as pspool,
        ):
            wqk_sb = wpool.tile([128, 512], f32r, tag="wqk")
            wv_sb = wpool.tile([128, 2048], f32r, tag="wv")
            for ci in range(4):
                nc.gpsimd.dma_start(wqk_sb[:, ci * 128:(ci + 1) * 128], wqk_in[ci])
                nc.gpsimd.dma_start(wv_sb[:, ci * 512:(ci + 1) * 512], wv_in[ci])

            for nb in range(16):  # 512-voxel blocks
                xt = xpool.tile([128, 2048], f32r, tag="x")
                for ci in range(4):
                    nc.gpsimd.dma_start(xt[:, ci * 512:(ci + 1) * 512],
                                        x_in[ci, :, nb * 512:(nb + 1) * 512])

                ps_qk = pspool.tile([128, 512], f32, tag="ps")
                for ci in range(4):
                    nc.tensor.matmul(ps_qk[:],
                                     wqk_sb[:, ci * 128:(ci + 1) * 128],
                                     xt[:, ci * 512:(ci + 1) * 512],
                                     start=(ci == 0), stop=(ci == 3))
                qk_sb = evpool.tile([128, 512], f32, tag="qk")
                nc.scalar.activation(qk_sb[:], ps_qk[:],
                                     mybir.ActivationFunctionType.Copy)
                nc.gpsimd.dma_start(qk_out[:, nb * 512:(nb + 1) * 512], qk_sb[:])

                for sub in range(4):  # 128-voxel sub-blocks -> vT
                    ps_v = pspool.tile([128, 512], f32, tag="ps")
                    for ci in range(4):
                        nc.tensor.matmul(
                            ps_v[:],
                            xt[:, ci * 512 + sub * 128:ci * 512 + (sub + 1) * 128],
                            wv_sb[:, ci * 512:(ci + 1) * 512],
                            start=(ci == 0), stop=(ci == 3))
                    v_sb = evpool.tile([128, 512], bf16, tag="v")
                    if sub % 2 == 0:
                        nc.scalar.activation(v_sb[:], ps_v[:],
                                             mybir.ActivationFunctionType.Copy)
                    else:
                        nc.vector.tensor_copy(v_sb[:], ps_v[:])
                    nc.gpsimd.dma_start(vt_out[nb * 4 + sub], v_sb[:])
    return nc


# --------------------------------------------------------------------------
# L2: energies + exp + per-line sums
# --------------------------------------------------------------------------
def build_l2():
    nc = bacc.Bacc()
    qs, ks, es, ss = {}, {}, {}, {}
    for ax in "hwd":
        qs[ax] = nc.declare_dram_parameter(f"q{ax}", [64, NV], f32, isOutput=False)
        ks[ax] = nc.declare_dram_parameter(f"k{ax}", [64, NV], f32, isOutput=False)
        es[ax] = nc.declare_dram_parameter(f"e{ax}", [128, 2048], bf16, isOutput=True)
        ss[ax] = nc.declare_dram_parameter(f"s{ax}", [128, 64], f32, isOutput=True)

    with tile.TileContext(nc) as tc:
        with (
            tc.tile_pool(name="qk", bufs=1) as qkpool,
            tc.tile_pool(name="ev", bufs=8) as evpool,
            tc.tile_pool(name="sm", bufs=1) as smpool,
            tc.tile_pool(name="ps", bufs=8, space="PSUM") as pspool,
        ):
            for ax in "hwd":
                q_sb = qkpool.tile([64, NV], f32, tag=f"q{ax}")
                k_sb = qkpool.tile([64, NV], f32, tag=f"k{ax}")
                nc.gpsimd.dma_start(q_sb[:], qs[ax][:])
                nc.gpsimd.dma_start(k_sb[:], ks[ax][:])
                s_sb = smpool.tile([128, 64], f32, tag=f"s{ax}")
                for bank in range(4):
                    ps = pspool.tile([128, 512], f32, tag="ps")
                    for q16 in range(16):
                        p = bank * 16 + q16
                        for j in range(4):
                            ln = 4 * p + j
                            nc.tensor.matmul(
                                ps[32 * j:32 * j + 32, q16 * 32:q16 * 32 + 32],
                                q_sb[:, ln * 32:ln * 32 + 32],
                                k_sb[:, ln * 32:ln * 32 + 32],
                                start=True, stop=True,
                                tile_position=(0, 32 * j))
                    e_sb = evpool.tile([128, 512], bf16, tag="e")
                    nc.scalar.activation(e_sb[:], ps[:],
                                         mybir.ActivationFunctionType.Exp)
                    nc.vector.tensor_reduce(
                        s_sb[:, bank * 16:bank * 16 + 16],
                        e_sb[:].rearrange("p (g l) -> p g l", l=32),
                        axis=mybir.AxisListType.X, op=mybir.AluOpType.add)
                    nc.gpsimd.dma_start(es[ax][:, bank * 512:(bank + 1) * 512], e_sb[:])
                nc.gpsimd.dma_start(ss[ax][:], s_sb[:])
    return nc


# --------------------------------------------------------------------------
# L3: aggregation with fused normalization
# --------------------------------------------------------------------------
def build_l3():
    nc = bacc.Bacc()
    as_, vs_, rs_, os_ = {}, {}, {}, {}
    for ax in "hwd":
        as_[ax] = nc.declare_dram_parameter(f"a{ax}", [128, 2048], bf16, isOutput=False)
        vs_[ax] = nc.declare_dram_parameter(f"v{ax}", [64, 128, 512], bf16, isOutput=False)
        rs_[ax] = nc.declare_dram_parameter(f"r{ax}", [128, 64], f32, isOutput=False)
        os_[ax] = nc.declare_dram_parameter(f"o{ax}", [64, 128, 512], bf16, isOutput=True)

    with tile.TileContext(nc) as tc:
        with (
            tc.tile_pool(name="aw", bufs=1) as apool,
            tc.tile_pool(name="vt", bufs=8) as vpool,
            tc.tile_pool(name="ev", bufs=8) as evpool,
            tc.tile_pool(name="ps", bufs=8, space="PSUM") as pspool,
        ):
            for ax in "hwd":
                a_sb = apool.tile([128, 2048], bf16, tag=f"a{ax}")
                r_sb = apool.tile([128, 64], f32, tag=f"r{ax}")
                nc.gpsimd.dma_start(a_sb[:], as_[ax][:])
                nc.gpsimd.dma_start(r_sb[:], rs_[ax][:])
                for p in range(PACKS):
                    v_sb = vpool.tile([128, 512], bf16, tag="v")
                    nc.gpsimd.dma_start(v_sb[:], vs_[ax][p])
                    ps = pspool.tile([128, 512], f32, tag="ps")
                    for j in range(4):
                        nc.tensor.matmul(
                            ps[32 * j:32 * j + 32, :],
                            a_sb[32 * j:32 * j + 32, p * 32:p * 32 + 32],
                            v_sb[32 * j:32 * j + 32, :],
                            start=True, stop=True,
                            tile_position=(32 * j, 32 * j))
                    o_sb = evpool.tile([128, 512], bf16, tag="o")
                    if p % 2 == 0:
                        nc.scalar.activation(o_sb[:], ps[:],
                                             mybir.ActivationFunctionType.Copy,
                                             scale=r_sb[:, p:p + 1])
                    else:
                        nc.vector.tensor_scalar_mul(o_sb[:], ps[:], r_sb[:, p:p + 1])
                    nc.gpsimd.dma_start(os_[ax][p], o_sb[:])
    return nc


def _get(name, builder):
    if name not in _cache:
        nc = builder()
        nc.finalize()
        _cache[name] = nc
    return _cache[name]


class _Runner:
    """jit-once PJRT runner for a prebuilt Bass module across 8 cores."""

    def __init__(self, nc):
        import jax
        from jax.experimental.shard_map import shard_map
        from jax.sharding import Mesh, PartitionSpec
        from concourse import bass2jax, mybir as _mb
        bass2jax.install_neuronx_cc_hook()
        self.nc = nc
        pname = nc.partition_id_tensor.name if nc.partition_id_tensor else None
        in_names, out_names, out_avals = [], [], []
        for alloc in nc.m.functions[0].allocations:
            if not isinstance(alloc, _mb.MemoryLocationSet):
                continue
            name = alloc.memorylocations[0].name
            if alloc.kind == "ExternalInput":
                if name != pname:
                    in_names.append(name)
            elif alloc.kind == "ExternalOutput":
                shape = tuple(alloc.tensor_shape)
                dt_np = _mb.dt.np(alloc.dtype)
                out_names.append(name)
                out_avals.append(jax.core.ShapedArray(shape, dt_np))
        self.in_names, self.out_names, self.out_avals = in_names, out_names, out_avals
        n_params = len(in_names)
        all_in = list(in_names) + list(out_names) + ([pname] if pname else [])

        def _body(*args):
            ops = list(args)
            if pname is not None:
                ops.append(bass2jax.partition_id_tensor())
            outs = bass2jax._bass_exec_p.bind(
                *ops, out_avals=tuple(out_avals), in_names=tuple(all_in),
                out_names=tuple(out_names), lowering_input_output_aliases=(),
                sim_require_finite=True, sim_require_nnan=True, nc=nc)
            return tuple(outs)

        devices = jax.devices()[:NCORES]
        mesh = Mesh(np.array(devices), ("core",))
        self.mesh = mesh
        n_io = n_params + len(out_names)
        self.donate = tuple(range(n_params, n_io))
        self.sharded = jax.jit(
            shard_map(_body, mesh=mesh,
                      in_specs=(PartitionSpec("core"),) * n_io,
                      out_specs=(PartitionSpec("core"),) * len(out_names),
                      check_rep=False),
            donate_argnums=self.donate, keep_unused=True)

    def _zeros(self):
        return [np.zeros((NCORES * a.shape[0], *a.shape[1:]), a.dtype)
                for a in self.out_avals]

    def __call__(self, in_maps):
        concat = [np.concatenate([np.asarray(m[n]) for m in in_maps], axis=0)
                  for n in self.in_names]
        arrs = self.sharded(*concat, *self._zeros())
        out = [{n: np.asarray(arrs[i]).reshape(NCORES, *self.out_avals[i].shape)[c]
                for i, n in enumerate(self.out_names)} for c in range(NCORES)]
        return out, (concat,)

    def bench(self, concat, n=3):
        import time, jax
        from jax.sharding import NamedSharding, PartitionSpec
        sh = NamedSharding(self.mesh, PartitionSpec("core"))
        dev_in = [jax.device_put(c, sh) for c in concat]
        for a in dev_in:
            a.block_until_ready()
        ts = []
        for _ in range(n):
            zs = [jax.device_put(z, sh) for z in self._zeros()]
            for z in zs:
                z.block_until_ready()
            t0 = time.perf_counter()
            arrs = self.sharded(*dev_in, *zs)
            for a in arrs:
                a.block_until_ready()
            ts.append(time.perf_counter() - t0)
        return min(ts)


class _RunRes:
    def __init__(self, results, exec_time_ns):
        self.results = results
        self.exec_time_ns = exec_time_ns


def _run(nc, in_maps, trace=False):
    import os
    key = id(nc)
    if key not in _cache:
        _cache[key] = _Runner(nc)
    runner = _cache[key]
    results, (concat,) = runner(in_maps)
    t = None
    if os.environ.get("BENCH"):
        t = int(runner.bench(concat, int(os.environ["BENCH"])) * 1e9)
    return _RunRes(results, t)


# --------------------------------------------------------------------------
# host orchestration
# --------------------------------------------------------------------------
def kernel(x, Wq, bq, Wk, bk, Wv, bv, gamma, _trace=False, _times=None):
    x = np.asarray(x, np.float32)
    Wq = np.asarray(Wq, np.float32); bq = np.asarray(bq, np.float32)
    Wk = np.asarray(Wk, np.float32); bk = np.asarray(bk, np.float32)
    Wv = np.asarray(Wv, np.float32); bv = np.asarray(bv, np.float32)
    gam = float(np.asarray(gamma))

    # ---------------- L1 ----------------
    wqk = np.concatenate([Wq.T, Wk.T], axis=1).reshape(4, 128, 128)
    wv = np.ascontiguousarray(Wv.T).reshape(4, 128, 512)
    in1 = []
    for core in range(NCORES):
        b, j = divmod(core, G)
        xc = x[b].reshape(C, H * W * D)[:, j * NV:(j + 1) * NV]
        in1.append({"x": np.ascontiguousarray(xc).reshape(4, 128, NV),
                    "wqk": wqk, "wv": wv})
    r1 = _run(_get("l1", build_l1), in1, trace=_trace)
    if _times is not None:
        _times.append(r1.exec_time_ns)

    q = np.empty((B, CQK, H * W * D), np.float32)
    k = np.empty((B, CQK, H * W * D), np.float32)
    vt = np.empty((B, H * W * D, 512), BF16)
    for core in range(NCORES):
        b, j = divmod(core, G)
        qk_c = r1.results[core]["qk"]
        q[b, :, j * NV:(j + 1) * NV] = qk_c[:64]
        k[b, :, j * NV:(j + 1) * NV] = qk_c[64:]
        vt[b, j * NV:(j + 1) * NV] = r1.results[core]["vt"].reshape(NV, 512)
    if bq.any():
        q += bq[None, :, None]
    if bk.any():
        k += bk[None, :, None]
    if bv.any():
        vt = (vt.astype(np.float32) + bv[None, None, :]).astype(BF16)

    # ---------------- L2 ----------------
    q4 = q.reshape(B, CQK, H, W, D)
    k4 = k.reshape(B, CQK, H, W, D)
    in2 = []
    for core in range(NCORES):
        b, g = divmod(core, G)
        sl = slice(g * DS, (g + 1) * DS)
        m = {}
        for nm, a4 in (("q", q4), ("k", k4)):
            m[nm + "h"] = np.ascontiguousarray(
                a4[b][:, :, :, sl].transpose(0, 2, 3, 1)).reshape(64, NV)
            m[nm + "w"] = np.ascontiguousarray(
                a4[b][:, :, :, sl].transpose(0, 1, 3, 2)).reshape(64, NV)
            m[nm + "d"] = np.ascontiguousarray(a4[b][:, sl]).reshape(64, NV)
        in2.append(m)
    r2 = _run(_get("l2", build_l2), in2, trace=_trace)
    if _times is not None:
        _times.append(r2.exec_time_ns)

    def dec_e(e):   # [128,2048] -> [256 lines, 32 q, 32 l]
        return np.ascontiguousarray(
            e.reshape(4, 32, 64, 32).transpose(2, 0, 1, 3)).reshape(LINES, 32, 32)

    def dec_s(s):   # [128,64] -> [256 lines, 32 q]
        return np.ascontiguousarray(
            s.reshape(4, 32, 64).transpose(2, 0, 1)).reshape(LINES, 32)

    ar = np.arange(32)
    E = {}          # (core, ax) -> masked exp energies [lines, q, l] float32
    sig = np.empty((B, H, W, D), np.float32)
    sig[:] = 0.0
    for core in range(NCORES):
        b, g = divmod(core, G)
        sl = slice(g * DS, (g + 1) * DS)
        for ax in "hwd":
            e = dec_e(r2.results[core][f"e{ax}"]).astype(np.float32)
            s = dec_s(r2.results[core][f"s{ax}"])
            if ax != "w":   # mask self: subtract diag from sums, zero diag
                s = s - e[:, ar, ar]
                e[:, ar, ar] = 0.0
            E[(core, ax)] = e
            if ax == "h":   # lines (w,dh), q=h
                sig[b, :, :, sl] += s.reshape(W, DS, 32).transpose(2, 0, 1)
            elif ax == "w":  # lines (h,dh), q=w
                sig[b, :, :, sl] += s.reshape(H, DS, 32).transpose(0, 2, 1)
            else:           # lines (h in slab, w), q=d
                sig[b, sl] += s.reshape(DS, W, 32)
    r = gam / sig   # [B, H, W, D]

    def pack_a(e):  # [lines, q, l] -> lhsT layout [128, 2048] bf16
        return np.ascontiguousarray(
            e.transpose(0, 2, 1).reshape(PACKS, 4, 32, 32)
            .transpose(1, 2, 0, 3)).reshape(128, 2048).astype(BF16)

    def pack_r(rv):  # [lines, q] -> [128, 64] f32
        return np.ascontiguousarray(
            rv.reshape(PACKS, 4, 32).transpose(1, 2, 0)).reshape(128, 64)

    vt4 = vt.reshape(B, H, W, D, 512)
    in3 = []
    for core in range(NCORES):
        b, g = divmod(core, G)
        sl = slice(g * DS, (g + 1) * DS)
        m = {}
        m["ah"] = pack_a(E[(core, "h")])
        m["aw"] = pack_a(E[(core, "w")])
        m["ad"] = pack_a(E[(core, "d")])
        m["rh"] = pack_r(np.ascontiguousarray(
            r[b][:, :, sl].transpose(1, 2, 0)).reshape(LINES, 32))
        m["rw"] = pack_r(np.ascontiguousarray(
            r[b][:, :, sl].transpose(0, 2, 1)).reshape(LINES, 32))
        m["rd"] = pack_r(r[b][sl].reshape(LINES, 32))
        m["vh"] = np.ascontiguousarray(
            vt4[b][:, :, sl].transpose(1, 2, 0, 3)).reshape(64, 128, 512)
        m["vw"] = np.ascontiguousarray(
            vt4[b][:, :, sl].transpose(0, 2, 1, 3)).reshape(64, 128, 512)
        m["vd"] = np.ascontiguousarray(vt4[b][sl]).reshape(64, 128, 512)
        in3.append(m)
    r3 = _run(_get("l3", build_l3), in3, trace=_trace)
    if _times is not None:
        _times.append(r3.exec_time_ns)

    # ---------------- final scatter-add ----------------
    acc = np.zeros((B, H, W, D, C), np.float32)
    for core in range(NCORES):
        b, g = divmod(core, G)
        sl = slice(g * DS, (g + 1) * DS)
        oh = r3.results[core]["oh"].astype(np.float32).reshape(PACKS, 4, 32, 512)
        ow = r3.results[core]["ow"].astype(np.float32).reshape(PACKS, 4, 32, 512)
        od = r3.results[core]["od"].astype(np.float32).reshape(PACKS, 4, 32, 512)
        # [pack, jj, q, c] -> [line, q, c]
        oh = oh.transpose(0, 1, 2, 3).reshape(LINES, 32, 512)
        ow = ow.reshape(LINES, 32, 512)
        od = od.reshape(LINES, 32, 512)
        acc[b][:, :, sl] += oh.reshape(W, DS, 32, 512).transpose(2, 0, 1, 3)
        acc[b][:, :, sl] += ow.reshape(H, DS, 32, 512).transpose(0, 2, 1, 3)
        acc[b][sl] += od.reshape(DS, W, 32, 512)
    y = x + acc.transpose(0, 4, 1, 2, 3)
    return y


# revision 15
# speedup vs baseline: 1.2609x; 1.0017x over previous
"""CrissCrossAttention3D Trainium2 kernel.

B=2, C=512, CQK=64, H=W=D=32, 8 NeuronCores.

Three SPMD launches (same program on all 8 cores, per-core data via in_maps),
host numpy resharding between launches:

  L1 (voxel-sharded, 8192 voxels/core):
      q,k = Wqk @ x   (psum [128, n]),  vT[n, c] = x_chunk.T-stationary @ WvT
  L2 (d-slab + h-slab roles per core): per-line energy matmuls
      E[q, l] = Q_line.T @ K_line  (fp32), exp on ACT -> bf16, per-line
      partial sums via DVE reduce.  No masking on device (host fixes diag).
  L3: aggregation  out[q, c] = sum_l a[l, q] * vT[l, c]  with 4-line
      diagonal 32x32 PE tiling, softmax normalization (and gamma) fused
      into the PSUM-evacuation scale.

Host: builds line-ordered views, computes r = gamma / sum(exp), zeroes the
masked diagonals, does the final scatter-add  y = x + oH + oW + oD.
"""

import numpy as np
import ml_dtypes

import concourse.bass as bass
from concourse import bacc
import concourse.tile as tile
from concourse import mybir
from concourse.bass_utils import run_bass_kernel_spmd

BF16 = ml_dtypes.bfloat16
B, C, H, W, D = 2, 512, 32, 32, 32
CQK = 64
NCORES = 8
G = 4          # d-groups (and h-groups for the D-axis role)
DS = D // G    # 8 slab thickness
NV = 8192      # voxels per core in every launch
LINES = 256    # lines per axis per core (H/W: 32*8, D: 8*32)
PACKS = 64     # 4-line packs per axis

f32 = mybir.dt.float32
f32r = mybir.dt.float32r
bf16 = mybir.dt.bfloat16

_cache = {}


# --------------------------------------------------------------------------
# L1: projections
# --------------------------------------------------------------------------
def build_l1():
    nc = bacc.Bacc()
    x_in = nc.declare_dram_parameter("x", [4, 128, NV], f32r, isOutput=False)
    wqk_in = nc.declare_dram_parameter("wqk", [4, 128, 128], f32r, isOutput=False)
    wv_in = nc.declare_dram_parameter("wv", [4, 128, 512], f32r, isOutput=False)
    qk_out = nc.declare_dram_parameter("qk", [128, NV], f32, isOutput=True)
    vt_out = nc.declare_dram_parameter("vt", [64, 128, 512], bf16, isOutput=True)

    with tile.TileContext(nc) as tc:
        with (
            tc.tile_pool(name="w", bufs=1) as wpool,
            tc.tile_pool(name="xb", bufs=2) as xpool,
            tc.tile_pool(name="ev", bufs=4) as evpool,
            tc.tile_pool(name="ps", bufs=4, space="PSUM") as pspool,
        ):
            wqk_sb = wpool.tile([128, 512], f32r, tag="wqk")
            wv_sb = wpool.tile([128, 2048], f32r, tag="wv")
            for ci in range(4):
                nc.gpsimd.dma_start(wqk_sb[:, ci * 128:(ci + 1) * 128], wqk_in[ci])
                nc.gpsimd.dma_start(wv_sb[:, ci * 512:(ci + 1) * 512], wv_in[ci])

            for nb in range(16):  # 512-voxel blocks
                xt = xpool.tile([128, 2048], f32r, tag="x")
                for ci in range(4):
                    nc.gpsimd.dma_start(xt[:, ci * 512:(ci + 1) * 512],
                                        x_in[ci, :, nb * 512:(nb + 1) * 512])

                ps_qk = pspool.tile([128, 512], f32, tag="ps")
                for ci in range(4):
                    nc.tensor.matmul(ps_qk[:],
                                     wqk_sb[:, ci * 128:(ci + 1) * 128],
                                     xt[:, ci * 512:(ci + 1) * 512],
                                     start=(ci == 0), stop=(ci == 3))
                qk_sb = evpool.tile([128, 512], f32, tag="qk")
                nc.scalar.activation(qk_sb[:], ps_qk[:],
                                     mybir.ActivationFunctionType.Copy)
                nc.gpsimd.dma_start(qk_out[:, nb * 512:(nb + 1) * 512], qk_sb[:])

                for sub in range(4):  # 128-voxel sub-blocks -> vT
                    ps_v = pspool.tile([128, 512], f32, tag="ps")
                    for ci in range(4):
                        nc.tensor.matmul(
                            ps_v[:],
                            xt[:, ci * 512 + sub * 128:ci * 512 + (sub + 1) * 128],
                            wv_sb[:, ci * 512:(ci + 1) * 512],
                            start=(ci == 0), stop=(ci == 3))
                    v_sb = evpool.tile([128, 512], bf16, tag="v")
                    if sub % 2 == 0:
                        nc.scalar.activation(v_sb[:], ps_v[:],
                                             mybir.ActivationFunctionType.Copy)
                    else:
                        nc.vector.tensor_copy(v_sb[:], ps_v[:])
                    nc.gpsimd.dma_start(vt_out[nb * 4 + sub], v_sb[:])
    return nc


# --------------------------------------------------------------------------
# L2: energies + exp + per-line sums
# --------------------------------------------------------------------------
def build_l2():
    nc = bacc.Bacc()
    qs, ks, es, ss = {}, {}, {}, {}
    for ax in "hwd":
        qs[ax] = nc.declare_dram_parameter(f"q{ax}", [64, NV], f32, isOutput=False)
        ks[ax] = nc.declare_dram_parameter(f"k{ax}", [64, NV], f32, isOutput=False)
        es[ax] = nc.declare_dram_parameter(f"e{ax}", [128, 2048], bf16, isOutput=True)
        ss[ax] = nc.declare_dram_parameter(f"s{ax}", [128, 64], f32, isOutput=True)

    with tile.TileContext(nc) as tc:
        with (
            tc.tile_pool(name="qk", bufs=1) as qkpool,
            tc.tile_pool(name="ev", bufs=8) as evpool,
            tc.tile_pool(name="sm", bufs=1) as smpool,
            tc.tile_pool(name="ps", bufs=8, space="PSUM") as pspool,
        ):
            for ax in "hwd":
                q_sb = qkpool.tile([64, NV], f32, tag=f"q{ax}")
                k_sb = qkpool.tile([64, NV], f32, tag=f"k{ax}")
                nc.gpsimd.dma_start(q_sb[:], qs[ax][:])
                nc.gpsimd.dma_start(k_sb[:], ks[ax][:])
                s_sb = smpool.tile([128, 64], f32, tag=f"s{ax}")
                for bank in range(4):
                    ps = pspool.tile([128, 512], f32, tag="ps")
                    for q16 in range(16):
                        p = bank * 16 + q16
                        for j in range(4):
                            ln = 4 * p + j
                            nc.tensor.matmul(
                                ps[32 * j:32 * j + 32, q16 * 32:q16 * 32 + 32],
                                q_sb[:, ln * 32:ln * 32 + 32],
                                k_sb[:, ln * 32:ln * 32 + 32],
                                start=True, stop=True,
                                tile_position=(0, 32 * j))
                    e_sb = evpool.tile([128, 512], bf16, tag="e")
                    nc.scalar.activation(e_sb[:], ps[:],
                                         mybir.ActivationFunctionType.Exp)
                    nc.vector.tensor_reduce(
                        s_sb[:, bank * 16:bank * 16 + 16],
                        e_sb[:].rearrange("p (g l) -> p g l", l=32),
                        axis=mybir.AxisListType.X, op=mybir.AluOpType.add)
                    nc.gpsimd.dma_start(es[ax][:, bank * 512:(bank + 1) * 512], e_sb[:])
                nc.gpsimd.dma_start(ss[ax][:], s_sb[:])
    return nc


# --------------------------------------------------------------------------
# L3: aggregation with fused normalization
# --------------------------------------------------------------------------
def build_l3():
    nc = bacc.Bacc()
    as_, vs_, rs_, os_ = {}, {}, {}, {}
    for ax in "hwd":
        as_[ax] = nc.declare_dram_parameter(f"a{ax}", [128, 8192], bf16, isOutput=False)
        vs_[ax] = nc.declare_dram_parameter(f"v{ax}", [64, 128, 512], bf16, isOutput=False)
        rs_[ax] = nc.declare_dram_parameter(f"r{ax}", [128, 64], f32, isOutput=False)
        os_[ax] = nc.declare_dram_parameter(f"o{ax}", [64, 128, 512], bf16, isOutput=True)

    with tile.TileContext(nc) as tc:
        with (
            tc.tile_pool(name="aw", bufs=1) as apool,
            tc.tile_pool(name="vt", bufs=8) as vpool,
            tc.tile_pool(name="ev", bufs=8) as evpool,
            tc.tile_pool(name="ps", bufs=8, space="PSUM") as pspool,
        ):
            for ax in "hwd":
                a_sb = apool.tile([128, 8192], bf16, tag=f"a{ax}")
                r_sb = apool.tile([128, 64], f32, tag=f"r{ax}")
                nc.gpsimd.dma_start(a_sb[:], as_[ax][:])
                nc.gpsimd.dma_start(r_sb[:], rs_[ax][:])
                for p in range(PACKS):
                    v_sb = vpool.tile([128, 512], bf16, tag="v")
                    nc.gpsimd.dma_start(v_sb[:], vs_[ax][p])
                    ps = pspool.tile([128, 512], f32, tag="ps")
                    nc.tensor.matmul(ps[:], a_sb[:, p * 128:(p + 1) * 128],
                                     v_sb[:], start=True, stop=True)
                    o_sb = evpool.tile([128, 512], bf16, tag="o")
                    if p % 2 == 0:
                        nc.scalar.activation(o_sb[:], ps[:],
                                             mybir.ActivationFunctionType.Copy,
                                             scale=r_sb[:, p:p + 1])
                    else:
                        nc.vector.tensor_scalar_mul(o_sb[:], ps[:], r_sb[:, p:p + 1])
                    nc.gpsimd.dma_start(os_[ax][p], o_sb[:])
    return nc


def _get(name, builder):
    if name not in _cache:
        nc = builder()
        nc.finalize()
        _cache[name] = nc
    return _cache[name]


class _Runner:
    """jit-once PJRT runner for a prebuilt Bass module across 8 cores."""

    def __init__(self, nc):
        import jax
        from jax.experimental.shard_map import shard_map
        from jax.sharding import Mesh, PartitionSpec
        from concourse import bass2jax, mybir as _mb
        bass2jax.install_neuronx_cc_hook()
        self.nc = nc
        pname = nc.partition_id_tensor.name if nc.partition_id_tensor else None
        in_names, out_names, out_avals = [], [], []
        for alloc in nc.m.functions[0].allocations:
            if not isinstance(alloc, _mb.MemoryLocationSet):
                continue
            name = alloc.memorylocations[0].name
            if alloc.kind == "ExternalInput":
                if name != pname:
                    in_names.append(name)
            elif alloc.kind == "ExternalOutput":
                shape = tuple(alloc.tensor_shape)
                dt_np = _mb.dt.np(alloc.dtype)
                out_names.append(name)
                out_avals.append(jax.core.ShapedArray(shape, dt_np))
        self.in_names, self.out_names, self.out_avals = in_names, out_names, out_avals
        n_params = len(in_names)
        all_in = list(in_names) + list(out_names) + ([pname] if pname else [])

        def _body(*args):
            ops = list(args)
            if pname is not None:
                ops.append(bass2jax.partition_id_tensor())
            outs = bass2jax._bass_exec_p.bind(
                *ops, out_avals=tuple(out_avals), in_names=tuple(all_in),
                out_names=tuple(out_names), lowering_input_output_aliases=(),
                sim_require_finite=True, sim_require_nnan=True, nc=nc)
            return tuple(outs)

        devices = jax.devices()[:NCORES]
        mesh = Mesh(np.array(devices), ("core",))
        self.mesh = mesh
        n_io = n_params + len(out_names)
        self.donate = tuple(range(n_params, n_io))
        self.sharded = jax.jit(
            shard_map(_body, mesh=mesh,
                      in_specs=(PartitionSpec("core"),) * n_io,
                      out_specs=(PartitionSpec("core"),) * len(out_names),
                      check_rep=False),
            donate_argnums=self.donate, keep_unused=True)

    def _zeros(self):
        return [np.zeros((NCORES * a.shape[0], *a.shape[1:]), a.dtype)
                for a in self.out_avals]

    def __call__(self, in_maps):
        concat = [np.concatenate([np.asarray(m[n]) for m in in_maps], axis=0)
                  for n in self.in_names]
        arrs = self.sharded(*concat, *self._zeros())
        out = [{n: np.asarray(arrs[i]).reshape(NCORES, *self.out_avals[i].shape)[c]
                for i, n in enumerate(self.out_names)} for c in range(NCORES)]
        return out, (concat,)

    def bench(self, concat, n=3):
        import time, jax
        from jax.sharding import NamedSharding, PartitionSpec
        sh = NamedSharding(self.mesh, PartitionSpec("core"))
        dev_in = [jax.device_put(c, sh) for c in concat]
        for a in dev_in:
            a.block_until_ready()
        ts = []
        for _ in range(n):
            zs = [jax.device_put(z, sh) for z in self._zeros()]
            for z in zs:
                z.block_until_ready()
            t0 = time.perf_counter()
            arrs = self.sharded(*dev_in, *zs)
            for a in arrs:
                a.block_until_ready()
            ts.append(time.perf_counter() - t0)
        return min(ts)


class _RunRes:
    def __init__(self, results, exec_time_ns):
        self.results = results
        self.exec_time_ns = exec_time_ns


def _run(nc, in_maps, trace=False):
    import os
    key = id(nc)
    if key not in _cache:
        _cache[key] = _Runner(nc)
    runner = _cache[key]
    results, (concat,) = runner(in_maps)
    t = None
    if os.environ.get("BENCH"):
        t = int(runner.bench(concat, int(os.environ["BENCH"])) * 1e9)
    return _RunRes(results, t)


# --------------------------------------------------------------------------
# host orchestration
# --------------------------------------------------------------------------
def kernel(x, Wq, bq, Wk, bk, Wv, bv, gamma, _trace=False, _times=None):
    x = np.asarray(x, np.float32)
    Wq = np.asarray(Wq, np.float32); bq = np.asarray(bq, np.float32)
    Wk = np.asarray(Wk, np.float32); bk = np.asarray(bk, np.float32)
    Wv = np.asarray(Wv, np.float32); bv = np.asarray(bv, np.float32)
    gam = float(np.asarray(gamma))

    # ---------------- L1 ----------------
    wqk = np.concatenate([Wq.T, Wk.T], axis=1).reshape(4, 128, 128)
    wv = np.ascontiguousarray(Wv.T).reshape(4, 128, 512)
    in1 = []
    for core in range(NCORES):
        b, j = divmod(core, G)
        xc = x[b].reshape(C, H * W * D)[:, j * NV:(j + 1) * NV]
        in1.append({"x": np.ascontiguousarray(xc).reshape(4, 128, NV),
                    "wqk": wqk, "wv": wv})
    r1 = _run(_get("l1", build_l1), in1, trace=_trace)
    if _times is not None:
        _times.append(r1.exec_time_ns)

    q = np.empty((B, CQK, H * W * D), np.float32)
    k = np.empty((B, CQK, H * W * D), np.float32)
    vt = np.empty((B, H * W * D, 512), BF16)
    for core in range(NCORES):
        b, j = divmod(core, G)
        qk_c = r1.results[core]["qk"]
        q[b, :, j * NV:(j + 1) * NV] = qk_c[:64]
        k[b, :, j * NV:(j + 1) * NV] = qk_c[64:]
        vt[b, j * NV:(j + 1) * NV] = r1.results[core]["vt"].reshape(NV, 512)
    if bq.any():
        q += bq[None, :, None]
    if bk.any():
        k += bk[None, :, None]
    if bv.any():
        vt = (vt.astype(np.float32) + bv[None, None, :]).astype(BF16)

    # ---------------- L2 ----------------
    q4 = q.reshape(B, CQK, H, W, D)
    k4 = k.reshape(B, CQK, H, W, D)
    in2 = []
    for core in range(NCORES):
        b, g = divmod(core, G)
        sl = slice(g * DS, (g + 1) * DS)
        m = {}
        for nm, a4 in (("q", q4), ("k", k4)):
            m[nm + "h"] = np.ascontiguousarray(
                a4[b][:, :, :, sl].transpose(0, 2, 3, 1)).reshape(64, NV)
            m[nm + "w"] = np.ascontiguousarray(
                a4[b][:, :, :, sl].transpose(0, 1, 3, 2)).reshape(64, NV)
            m[nm + "d"] = np.ascontiguousarray(a4[b][:, sl]).reshape(64, NV)
        in2.append(m)
    r2 = _run(_get("l2", build_l2), in2, trace=_trace)
    if _times is not None:
        _times.append(r2.exec_time_ns)

    def dec_e(e):   # [128,2048] -> [256 lines, 32 q, 32 l]
        return np.ascontiguousarray(
            e.reshape(4, 32, 64, 32).transpose(2, 0, 1, 3)).reshape(LINES, 32, 32)

    def dec_s(s):   # [128,64] -> [256 lines, 32 q]
        return np.ascontiguousarray(
            s.reshape(4, 32, 64).transpose(2, 0, 1)).reshape(LINES, 32)

    ar = np.arange(32)
    E = {}          # (core, ax) -> masked exp energies [lines, q, l] float32
    sig = np.empty((B, H, W, D), np.float32)
    sig[:] = 0.0
    for core in range(NCORES):
        b, g = divmod(core, G)
        sl = slice(g * DS, (g + 1) * DS)
        for ax in "hwd":
            e = dec_e(r2.results[core][f"e{ax}"]).astype(np.float32)
            s = dec_s(r2.results[core][f"s{ax}"])
            if ax != "w":   # mask self: subtract diag from sums, zero diag
                s = s - e[:, ar, ar]
                e[:, ar, ar] = 0.0
            E[(core, ax)] = e
            if ax == "h":   # lines (w,dh), q=h
                sig[b, :, :, sl] += s.reshape(W, DS, 32).transpose(2, 0, 1)
            elif ax == "w":  # lines (h,dh), q=w
                sig[b, :, :, sl] += s.reshape(H, DS, 32).transpose(0, 2, 1)
            else:           # lines (h in slab, w), q=d
                sig[b, sl] += s.reshape(DS, W, 32)
    r = gam / sig   # [B, H, W, D]

    def pack_a(e):  # [lines, q, l] -> block-diag lhsT [128, PACKS*128] bf16
        eT = e.transpose(0, 2, 1).reshape(PACKS, 4, 32, 32)   # [p, jj, l, q]
        blk = np.zeros((PACKS, 4, 32, 4, 32), np.float32)
        for jj in range(4):
            blk[:, jj, :, jj, :] = eT[:, jj]
        return np.ascontiguousarray(
            blk.transpose(1, 2, 0, 3, 4)).reshape(128, PACKS * 128).astype(BF16)

    def pack_r(rv):  # [lines, q] -> [128, 64] f32
        return np.ascontiguousarray(
            rv.reshape(PACKS, 4, 32).transpose(1, 2, 0)).reshape(128, 64)

    vt4 = vt.reshape(B, H, W, D, 512)
    in3 = []
    for core in range(NCORES):
        b, g = divmod(core, G)
        sl = slice(g * DS, (g + 1) * DS)
        m = {}
        m["ah"] = pack_a(E[(core, "h")])
        m["aw"] = pack_a(E[(core, "w")])
        m["ad"] = pack_a(E[(core, "d")])
        m["rh"] = pack_r(np.ascontiguousarray(
            r[b][:, :, sl].transpose(1, 2, 0)).reshape(LINES, 32))
        m["rw"] = pack_r(np.ascontiguousarray(
            r[b][:, :, sl].transpose(0, 2, 1)).reshape(LINES, 32))
        m["rd"] = pack_r(r[b][sl].reshape(LINES, 32))
        m["vh"] = np.ascontiguousarray(
            vt4[b][:, :, sl].transpose(1, 2, 0, 3)).reshape(64, 128, 512)
        m["vw"] = np.ascontiguousarray(
            vt4[b][:, :, sl].transpose(0, 2, 1, 3)).reshape(64, 128, 512)
        m["vd"] = np.ascontiguousarray(vt4[b][sl]).reshape(64, 128, 512)
        in3.append(m)
    r3 = _run(_get("l3", build_l3), in3, trace=_trace)
    if _times is not None:
        _times.append(r3.exec_time_ns)

    # ---------------- final scatter-add ----------------
    acc = np.zeros((B, H, W, D, C), np.float32)
    for core in range(NCORES):
        b, g = divmod(core, G)
        sl = slice(g * DS, (g + 1) * DS)
        oh = r3.results[core]["oh"].astype(np.float32).reshape(PACKS, 4, 32, 512)
        ow = r3.results[core]["ow"].astype(np.float32).reshape(PACKS, 4, 32, 512)
        od = r3.results[core]["od"].astype(np.float32).reshape(PACKS, 4, 32, 512)
        # [pack, jj, q, c] -> [line, q, c]
        oh = oh.transpose(0, 1, 2, 3).reshape(LINES, 32, 512)
        ow = ow.reshape(LINES, 32, 512)
        od = od.reshape(LINES, 32, 512)
        acc[b][:, :, sl] += oh.reshape(W, DS, 32, 512).transpose(2, 0, 1, 3)
        acc[b][:, :, sl] += ow.reshape(H, DS, 32, 512).transpose(0, 2, 1, 3)
        acc[b][sl] += od.reshape(DS, W, 32, 512)
    y = x + acc.transpose(0, 4, 1, 2, 3)
    return y
